# revision 43
# baseline (speedup 1.0000x reference)
"""Trainium2 Bass kernel for a post-LN multi-head-attention block.

Reference computation (B=4, S=2048, D=1024, 16 heads x 64):
    q,k,v = x @ W{q,k,v}.T ; attn = softmax(q k^T/8 + mask) ; o = attn v
    out = LayerNorm(query + (o @ Wo.T)) * gamma + beta

Sharding: 8 cores = 4 batches x 2 query-halves (1024 query rows per core).
Each core computes all 16 heads for its query rows against the full
(mask-compacted) key set of its batch.  No collectives.

Key implementation choices:
  - keys with mask==0 contribute exactly 0 attention weight, so the host
    compacts key/value to the unmasked subset, padded to a multiple of 128
    (padding biased -1e30 so exp -> 0).
  - Q/K/V/O projections and attnV run in fp8e4m3 with DoubleRow perf
    mode (2 contraction rows per PE pass = 2x f32r/bf16 matmul
    throughput).  The attention output is only ~5% of the residual
    magnitude entering LayerNorm, so fp8 noise dilutes ~20x in the final
    output (measured end-to-end rel err ~9e-3 vs the 2e-2 budget).
  - scores keep fp16 operands (kts/qts drained straight from PSUM, no
    extra regroup DMAs; contraction is only 64 so DoubleRow would need a
    partition-regrouped layout whose DMA-issue cost exceeds the PE win).
  - exp bias includes -4.5 so fp8 e2 = exp(qk/8 - 4.5) stays below e4m3
    max 240 (scores reach ~9.3); the shift cancels in normalization.
  - scores are computed transposed, scoresT[k, q], so softmax's
    k-reduction becomes a matmul reduction: V is augmented with a
    ones-column and attnV produces [out^T ; rowsum] in one PSUM group.
  - attnV DoubleRow packs kt-pairs: e2 for kt and kt+1 land in one
    [128, 2, 2head, 512] fp8 tile that feeds rhs [128, 2, 512] views.
  - normalization (divide by rowsum) happens after attnV via a K=1
    broadcast matmul of 1/rowsum and an elementwise multiply.
  - loop nest is qc-outer / head-pair-inner: all projections fill the
    TensorE during the first query chunk's (ScalarE-bound) attention,
    and the output-projection + LayerNorm for chunk 0 overlaps chunk
    1's attention, leaving only half the epilogue as a serial tail.
"""

import numpy as np
import ml_dtypes

import concourse.bacc as bacc
import concourse.tile as tile
import concourse.bass as bass
from concourse import mybir
from concourse.bass_utils import run_bass_kernel_spmd

DMODEL = 1024
NHEAD = 16
HD = 64
B = 4
S = 2048
NCORES = 8
SQ = 1024          # query rows per core
P = 128
F32 = mybir.dt.float32
F32R = mybir.dt.float32r
F16 = mybir.dt.float16
F8 = mybir.dt.float8e4
DR = mybir.MatmulPerfMode.DoubleRow
NPF8 = ml_dtypes.float8_e4m3
ET = DMODEL // P   # 8 e-tiles (feature tiles)
DTL = DMODEL // P  # 8 d-tiles (contraction tiles)
NQC = SQ // 512    # 2 query chunks of 512
NEC = DMODEL // 512  # 2 feature chunks of 512
NPAIR = NHEAD // 2   # 8 head pairs; pair j = heads (2j, 2j+1) in e-tile j


def _balanced_chunks(total, maxw=512):
    """Split `total` (a multiple of 128) into maximal chunks <= maxw,
    widest first: 512B-contiguous DMA runs dodge the sub-512B 2x
    latency penalty."""
    out, lo = [], 0
    while lo < total:
        w = min(maxw, total - lo)
        out.append((lo, lo + w))
        lo += w
    return out


def _build(LPAD, do_compile=True, reps=1, phases=5):
    KT = LPAD // P
    KCH = _balanced_chunks(LPAD)
    QCH = _balanced_chunks(SQ)
    nc = bacc.Bacc("TRN2", target_bir_lowering=False, debug=False,
                   num_devices=NCORES)

    qT = nc.declare_dram_parameter("qT", [DMODEL, SQ], F8, isOutput=False)
    kT = nc.declare_dram_parameter("kT", [DMODEL, LPAD], F8, isOutput=False)
    vT = nc.declare_dram_parameter("vT", [DMODEL, LPAD], F8, isOutput=False)
    resid = nc.declare_dram_parameter("resid", [SQ, DMODEL], F32, isOutput=False)
    wqT = nc.declare_dram_parameter("wqT", [DMODEL, DMODEL], F8, isOutput=False)
    wkT = nc.declare_dram_parameter("wkT", [DMODEL, DMODEL], F8, isOutput=False)
    wvT = nc.declare_dram_parameter("wvT", [DMODEL, DMODEL], F8, isOutput=False)
    woT = nc.declare_dram_parameter("woT", [DMODEL, DMODEL], F8, isOutput=False)
    maskb = nc.declare_dram_parameter("maskb", [P, KT], F32, isOutput=False)
    gamma = nc.declare_dram_parameter("gamma", [DMODEL], F32, isOutput=False)
    beta = nc.declare_dram_parameter("beta", [DMODEL], F32, isOutput=False)
    out = nc.declare_dram_parameter("out", [SQ, DMODEL], F32, isOutput=True)

    def dram3(ap):
        # (o*P, width) DRAM tensor viewed as [p, o, width]
        return ap.rearrange("(o p) w -> p o w", p=P)

    with tile.TileContext(nc) as tc:
        with (
            tc.tile_pool(name="keep", bufs=1) as keep,      # long-lived SBUF
            tc.tile_pool(name="dpool", bufs=2) as dpool,
            tc.tile_pool(name="pproj", bufs=2, space="PSUM") as pproj,
            tc.tile_pool(name="pscore", bufs=2, space="PSUM") as pscore,
            tc.tile_pool(name="pattn", bufs=1, space="PSUM") as pattn,
        ):
            for _rep in range(reps):
                self_body(nc, tc, keep, dpool, pproj, pscore, pattn, phases,
                          LPAD, KT, KCH, QCH,
                          qT, kT, vT, resid, wqT, wkT, wvT, woT,
                          maskb, gamma, beta, out, dram3)
    if do_compile:
        nc.compile()
    return nc


def self_body(nc, tc, keep, dpool, pproj, pscore, pattn, phases, LPAD, KT,
              KCH, QCH,
              qT, kT, vT, resid, wqT, wkT, wvT, woT, maskb, gamma, beta,
              out, dram3):
    NKTP = KT // 2            # full DoubleRow kt-pairs in attnV
    ODD = KT % 2
    VH = 512
    NVP = DMODEL // VH        # V-projection weight slices
    HPS = VH // HD            # heads per slice

    # ---- long-lived tensors ----
    vaug = keep.tile([P, KT, NHEAD, HD + 1], F8)     # [k | head | V,1]
    aoT = keep.tile([P, ET, SQ], F8)                 # attn out^T (d' on part)
    maskb_sb = keep.tile([P, KT], F32)
    nc.sync.dma_start(out=maskb_sb, in_=maskb.ap())
    ones_sb = keep.tile([65, 64], F32R)
    nc.vector.memset(ones_sb[64:65, :].bitcast(F32), 1.0)
    ones16 = keep.tile([P, KT, NHEAD, 1], F32)
    nc.vector.memset(ones16, 1.0)
    nc.vector.tensor_copy(out=vaug[:, :, :, HD:HD + 1], in_=ones16)
    kts = [keep.tile([P, LPAD], F16, name=f"kts{j}")
           for j in range(NPAIR)]
    qts = [keep.tile([P, SQ], F16, name=f"qts{j}")
           for j in range(NPAIR)]

    vT3 = dram3(vT.ap())
    wvT3 = dram3(wvT.ap())
    kT3 = dram3(kT.ap())
    qT3 = dram3(qT.ap())

    # resident weights (fp8: 8KB/partition each) and projection inputs.
    # DMA order is the first-exp critical path: pair-0 weight columns and
    # the first kin/qin/vin chunks land before the bulk transfers, which
    # are themselves ordered by the fill-thunk deadlines that need them.
    wkT3 = dram3(wkT.ap())
    wqT3 = dram3(wqT.ap())
    wk_sb = keep.tile([P, DTL, DMODEL], F8)
    wq_sb = keep.tile([P, DTL, DMODEL], F8)
    wv_sb = keep.tile([P, DTL, DMODEL], F8)
    kin = keep.tile([P, DTL, LPAD], F8)
    qin = keep.tile([P, DTL, SQ], F8)
    vin = keep.tile([P, DTL, LPAD], F8)
    K0 = KCH[0][1]
    W0 = 512               # first weight slice: covers pairs 0-3
    nc.sync.dma_start(out=wk_sb[:, :, 0:W0], in_=wkT3[:, :, 0:W0])
    nc.sync.dma_start(out=kin[:, :, 0:K0], in_=kT3[:, :, 0:K0])
    nc.sync.dma_start(out=wq_sb[:, :, 0:W0], in_=wqT3[:, :, 0:W0])
    nc.sync.dma_start(out=qin[:, :, 0:QCH[0][1]],
                      in_=qT3[:, :, 0:QCH[0][1]])
    nc.sync.dma_start(out=wv_sb[:, :, 0:W0], in_=wvT3[:, :, 0:W0])
    nc.sync.dma_start(out=vin[:, :, 0:K0], in_=vT3[:, :, 0:K0])
    if K0 < LPAD:
        nc.sync.dma_start(out=kin[:, :, K0:LPAD], in_=kT3[:, :, K0:LPAD])
        nc.sync.dma_start(out=vin[:, :, K0:LPAD], in_=vT3[:, :, K0:LPAD])
    nc.sync.dma_start(out=wk_sb[:, :, W0:DMODEL], in_=wkT3[:, :, W0:DMODEL])
    nc.sync.dma_start(out=wq_sb[:, :, W0:DMODEL], in_=wqT3[:, :, W0:DMODEL])
    nc.sync.dma_start(out=wv_sb[:, :, W0:DMODEL], in_=wvT3[:, :, W0:DMODEL])
    for (lo, hi) in QCH[1:]:
        nc.sync.dma_start(out=qin[:, :, lo:hi], in_=qT3[:, :, lo:hi])

    def project_pair(j):
        """K^T and Q^T projection thunks for head-pair j (fp8 DoubleRow,
        drained straight to fp16 kts[j]/qts[j])."""
        thunks = []
        for (lo, hi) in KCH:
            def kthunk(lo=lo, hi=hi, j=j):
                w = hi - lo
                ps = pproj.tile([P, 512], F32, tag="pp")
                for dt in range(DTL // 2):
                    nc.tensor.matmul(
                        ps[:, :w],
                        lhsT=wk_sb[:, 2 * dt:2 * dt + 2, j * P:(j + 1) * P],
                        rhs=kin[:, 2 * dt:2 * dt + 2, lo:hi],
                        start=(dt == 0), stop=(dt == DTL // 2 - 1),
                        perf_mode=DR)
                nc.vector.tensor_copy(out=kts[j][:, lo:hi], in_=ps[:, :w])
            thunks.append(kthunk)
        for (lo, hi) in QCH:
            def qthunk(lo=lo, hi=hi, j=j):
                w = hi - lo
                ps = pproj.tile([P, 512], F32, tag="pp")
                for dt in range(DTL // 2):
                    nc.tensor.matmul(
                        ps[:, :w],
                        lhsT=wq_sb[:, 2 * dt:2 * dt + 2, j * P:(j + 1) * P],
                        rhs=qin[:, 2 * dt:2 * dt + 2, lo:hi],
                        start=(dt == 0), stop=(dt == DTL // 2 - 1),
                        perf_mode=DR)
                nc.vector.tensor_copy(out=qts[j][:, lo:hi], in_=ps[:, :w])
            thunks.append(qthunk)
        return thunks

    # V-projection per pair: kt-blocks grouped ~3 per thunk so a pair's
    # V work fits the same fill-slot budget as its K/Q projections
    _vstep = min((KT + 2) // 3, 4)
    VGRP = [(a, min(a + _vstep, KT)) for a in range(0, KT, _vstep)]

    def vproj_pair(j):
        thunks = []
        for (g0, g1) in VGRP:
            def vthunk(g0=g0, g1=g1, j=j):
                gl = g1 - g0
                ps = pproj.tile([P, 512], F32, tag="pp")
                for i in range(gl):
                    for dt in range(DTL // 2):
                        nc.tensor.matmul(
                            ps[:, i * P:(i + 1) * P],
                            lhsT=vin[:, 2 * dt:2 * dt + 2,
                                     (g0 + i) * P:(g0 + i + 1) * P],
                            rhs=wv_sb[:, 2 * dt:2 * dt + 2,
                                      j * P:(j + 1) * P],
                            start=(dt == 0), stop=(dt == DTL // 2 - 1),
                            perf_mode=DR)
                nc.vector.tensor_copy(
                    out=vaug[:, g0:g1, 2 * j:2 * j + 2, 0:HD],
                    in_=ps[:, 0:gl * P].rearrange(
                        "p (k h x) -> p k h x", h=2, x=HD))
            thunks.append(vthunk)
        return thunks

    # ---- phase E pieces (output proj + residual + layernorm);
    # wo DMA is issued after the prologue so it doesn't delay vin ----
    wo_sb = keep.tile([P, DTL, DMODEL], F8)
    gamma_sb = keep.tile([P, DMODEL], F32)
    beta_sb = keep.tile([P, DMODEL], F32)
    eps_sb = keep.tile([P, 1], F32)
    nc.vector.memset(eps_sb, 1e-5)
    resid3 = dram3(resid.ap())
    out3 = dram3(out.ap())

    # phase E is split in three so the batched Sqrt touches the ScalarE
    # activation table only twice per 4-block batch instead of
    # thrashing Exp<->Sqrt on every block
    NST = SQ // P
    mv_all = keep.tile([P, NST, 2], F32)     # per-block (mean, var)
    sd_all = keep.tile([P, NST, 2], F32)     # (sqrt(var+eps), rstd)
    xts = {}

    def stE_pre(st):
        """Output projection + residual + bn stats for query block st."""
        rin = rin_pre[:, st, :]
        x_t = dpool.tile([P, DMODEL], F32, tag="x", bufs=5)
        xts[st] = x_t
        for ec in range(NEC):
            ps = pproj.tile([P, 512], F32, tag="pp")
            for dj in range(DTL // 2):
                nc.tensor.matmul(
                    ps,
                    lhsT=aoT[:, 2 * dj:2 * dj + 2, st * P:(st + 1) * P],
                    rhs=wo_sb[:, 2 * dj:2 * dj + 2,
                              ec * 512:(ec + 1) * 512],
                    start=(dj == 0), stop=(dj == DTL // 2 - 1),
                    perf_mode=DR)
            nc.vector.tensor_add(
                out=x_t[:, ec * 512:(ec + 1) * 512],
                in0=ps, in1=rin[:, ec * 512:(ec + 1) * 512])
        stats = dpool.tile([P, 2, 6], F32, tag="stats", bufs=2)
        nc.vector.bn_stats(out=stats[:, 0, :], in_=x_t[:, 0:512])
        nc.vector.bn_stats(out=stats[:, 1, :], in_=x_t[:, 512:1024])
        nc.vector.bn_aggr(out=mv_all[:, st, :], in_=stats)

    def stE_sqrt(lo, hi):
        """Batched rstd for query blocks [lo, hi)."""
        nc.scalar.activation(out=sd_all[:, lo:hi, 0],
                             in_=mv_all[:, lo:hi, 1],
                             func=mybir.ActivationFunctionType.Sqrt,
                             bias=eps_sb[:, 0:1], scale=1.0)
        nc.vector.reciprocal(out=sd_all[:, lo:hi, 1],
                             in_=sd_all[:, lo:hi, 0])

    def stE_post(st):
        """LayerNorm application + store for query block st (normalize
        on DVE, gamma/beta on Pool so consecutive blocks pipeline)."""
        x_t = xts.pop(st)
        nc.vector.tensor_scalar(
            out=x_t, in0=x_t, scalar1=mv_all[:, st, 0:1],
            scalar2=sd_all[:, st, 1:2],
            op0=mybir.AluOpType.subtract, op1=mybir.AluOpType.mult)
        y_t = dpool.tile([P, DMODEL], F32, tag="y", bufs=2)
        nc.gpsimd.tensor_mul(out=y_t, in0=x_t, in1=gamma_sb)
        nc.gpsimd.tensor_add(out=y_t, in0=y_t, in1=beta_sb)
        nc.sync.dma_start(out=out3[:, st, :], in_=y_t)

    # ======== prologue: pair 0's projections (kthunk0/qthunk0 first:
    # they gate the first scores+exp) + pair 0's V columns ========
    thunks0 = project_pair(0)
    nk = len(KCH)
    vthunks0 = vproj_pair(0)
    order = [thunks0[0], thunks0[nk], vthunks0[0], thunks0[1]]
    order += thunks0[nk + 1:] + thunks0[2:nk] + vthunks0[1:]
    for t in order:
        t()
    nc.sync.dma_start(out=wo_sb, in_=dram3(woT.ap()))
    nc.gpsimd.dma_start(out=gamma_sb, in_=bass.AP(
        tensor=gamma.ap().tensor, offset=0, ap=[[0, P], [1, DMODEL]]))
    nc.gpsimd.dma_start(out=beta_sb, in_=bass.AP(
        tensor=beta.ap().tensor, offset=0, ap=[[0, P], [1, DMODEL]]))
    rin_pre = keep.tile([P, SQ // P, DMODEL], F32)
    for st in range(SQ // P):
        nc.sync.dma_start(out=rin_pre[:, st, :], in_=resid3[:, st, :])

    if phases < 4:
        for j in range(1, NPAIR):
            for t in project_pair(j) + vproj_pair(j):
                t()
        nc.sync.dma_start(out=out.ap()[0:P, 0:LPAD // 2],
                          in_=kts[NPAIR - 1].bitcast(F32))
        nc.sync.dma_start(out=out.ap()[P:2 * P, 0:SQ // 2],
                          in_=qts[NPAIR - 1].bitcast(F32))
        return

    # ======== main: qc outer, head-pair inner ========
    # qc0: remaining projections as TensorE filler.  qc1: phase E for
    # qc0's rows as filler.  Epilogues deferred one iteration so ScalarE
    # never waits at iteration boundaries.
    pend = [None]
    for qc in range(NQC):
        qsl = slice(qc * 512, (qc + 1) * 512)
        if qc == 1:
            # phase E for qc0's blocks, one piece per head-pair slot
            stqueue = [lambda s=s: stE_pre(s) for s in range(NST // 2)]
            stqueue.append(lambda: (stE_sqrt(0, NST // 2), stE_post(0)))
            stqueue += [lambda s=s: stE_post(s) for s in range(1, NST // 2)]
        for j in range(NPAIR):
            fill = []
            if qc == 0 and j + 1 < NPAIR:
                fill += project_pair(j + 1)
                fill += vproj_pair(j + 1)
            stfill = []
            if qc == 1 and stqueue:
                stfill = [stqueue.pop(0)]

            avA = pattn.tile([HD + 1, 512], F32, tag="avA")
            avB = pattn.tile([HD + 1, 512], F32, tag="avB")
            es = {}

            def scores(kt, es=es, j=j, qsl=qsl):
                ps = pscore.tile([P, 1024], F32, tag="ps_s")
                nc.tensor.matmul(
                    ps[:, 0:512],
                    lhsT=kts[j][0:64, kt * P:(kt + 1) * P],
                    rhs=qts[j][0:64, qsl], start=True, stop=True)
                nc.tensor.matmul(
                    ps[:, 512:1024],
                    lhsT=kts[j][64:128, kt * P:(kt + 1) * P],
                    rhs=qts[j][64:128, qsl], start=True, stop=True)
                ktp, par = divmod(kt, 2)
                if par == 0:
                    e2 = dpool.tile([P, 2, 2, 512], F8, tag="e2", bufs=3)
                    es[ktp] = e2
                else:
                    e2 = es[ktp]
                nc.scalar.activation(
                    out=e2[:, par, :, :],
                    in_=ps.rearrange("p (a b) -> p a b", b=512),
                    func=mybir.ActivationFunctionType.Exp,
                    bias=maskb_sb[:, kt:kt + 1], scale=0.125)

            def attnv(ktp, es=es, avA=avA, avB=avB, j=j):
                st = (ktp == 0)
                sp = (ktp == NKTP - 1) and not ODD
                if ktp < NKTP:
                    e2 = es[ktp]
                    nc.tensor.matmul(
                        avA, lhsT=vaug[:, 2 * ktp:2 * ktp + 2, 2 * j, :],
                        rhs=e2[:, :, 0, :],
                        start=st, stop=sp, perf_mode=DR)
                    nc.tensor.matmul(
                        avB, lhsT=vaug[:, 2 * ktp:2 * ktp + 2,
                                       2 * j + 1, :],
                        rhs=e2[:, :, 1, :],
                        start=st, stop=sp, perf_mode=DR)
                else:          # leftover single-kt pass (odd KT)
                    e2 = es[ktp]
                    nc.tensor.matmul(
                        avA, lhsT=vaug[:, KT - 1, 2 * j, :],
                        rhs=e2[:, 0, 0, :],
                        start=False, stop=True)
                    nc.tensor.matmul(
                        avB, lhsT=vaug[:, KT - 1, 2 * j + 1, :],
                        rhs=e2[:, 0, 1, :],
                        start=False, stop=True)

            # software pipeline: scores(kt) ahead of attnv;
            # previous iteration's epilogue lands after scores(0)
            scores(0)
            if pend[0] is not None:
                pend[0]()          # prev iteration's attnv tail + epilogue
                pend[0] = None
            for kt in range(1, KT):
                scores(kt)
                if kt % 2 == 1 and kt >= 3:
                    attnv(kt // 2 - 1)
                if fill:
                    fill.pop(0)()
                elif stfill and kt >= 4:
                    stfill.pop(0)()
            # pair j+1's projections must land before its attention:
            # flush any fills the kt slots didn't cover
            while fill:
                fill.pop(0)()

            def epilogue(avA=avA, avB=avB, j=j, qsl=qsl,
                         attnv=attnv, es=es):
                # deferred pipeline tail
                if NKTP > 0:
                    attnv(NKTP - 1)
                if ODD:
                    attnv(NKTP)
                # normalize in place: aoT = av / rowsum, reading the
                # PSUM accumulators directly (row 64 is the rowsum)
                recip_t = dpool.tile([65, 1024], F32R, tag="recip",
                                     bufs=2)
                with nc.allow_low_precision(
                        reason="f32r recip feeds f32r matmul"):
                    nc.vector.reciprocal(
                        out=recip_t[64:65, 0:512], in_=avA[64:65, :])
                    nc.vector.reciprocal(
                        out=recip_t[64:65, 512:1024], in_=avB[64:65, :])
                rbA = pproj.tile([P, 512], F32, tag="pp")
                rbB = pproj.tile([P, 512], F32, tag="pp")
                nc.tensor.matmul(rbA[0:64, :], lhsT=ones_sb[64:65, :],
                                 rhs=recip_t[64:65, 0:512],
                                 start=True, stop=True)
                nc.tensor.matmul(rbB[0:64, :], lhsT=ones_sb[64:65, :],
                                 rhs=recip_t[64:65, 512:1024],
                                 start=True, stop=True)
                # DVE instructions may read only one PSUM operand:
                # drain the broadcast rows to SBUF (recip_t rows 0:64)
                nc.vector.tensor_copy(out=recip_t[0:64, 0:512],
                                      in_=rbA[0:64, :])
                nc.vector.tensor_copy(out=recip_t[0:64, 512:1024],
                                      in_=rbB[0:64, :])
                nc.vector.tensor_tensor(
                    aoT[0:64, j, qsl], avA[0:64, :],
                    recip_t[0:64, 0:512].bitcast(F32),
                    mybir.AluOpType.mult)
                av_f8 = dpool.tile([64, 512], F8, tag="av_f8", bufs=2)
                nc.vector.tensor_tensor(
                    av_f8, avB[0:64, :],
                    recip_t[0:64, 512:1024].bitcast(F32),
                    mybir.AluOpType.mult)
                nc.sync.dma_start(out=aoT[64:128, j, qsl], in_=av_f8)

            pend[0] = epilogue

    if pend[0] is not None:
        pend[0]()
        pend[0] = None

    if phases < 5:
        return

    # phase E tail: qc1's query blocks (qc0's ran as qc1 attention
    # fills).  Per-block sqrt chains here: ScalarE is idle, so act-table
    # switches are free and the blocks pipeline across engines.
    for st in range(NST // 2, NST):
        stE_pre(st)
        stE_sqrt(st, st + 1)
        stE_post(st)


_cache = {}


def _get_nc(LPAD):
    if LPAD not in _cache:
        _cache[LPAD] = _build(LPAD)
    return _cache[LPAD]


def make_in_maps(query, key, value, mask, Wq, Wk, Wv, Wo, ln_gamma, ln_beta):
    """Host-side sharding: returns (in_maps, LPAD)."""
    f = lambda a: np.ascontiguousarray(np.asarray(a, np.float32))
    f8 = lambda a: np.ascontiguousarray(
        np.asarray(np.asarray(a, np.float32), NPF8))
    query, key, value = f(query), f(key), f(value)
    mask = np.asarray(mask)
    wqT = f8(np.asarray(Wq, np.float32).T)
    wkT = f8(np.asarray(Wk, np.float32).T)
    wvT = f8(np.asarray(Wv, np.float32).T)
    woT = f8(np.asarray(Wo, np.float32).T)
    gamma, beta = f(ln_gamma), f(ln_beta)

    idxs = []
    for b in range(B):
        ix = np.nonzero(mask[b] != 0)[0]
        if len(ix) == 0:
            # all-masked row: the -1e8 bias is common to every key, so the
            # reference softmax reduces to plain softmax over all keys.
            ix = np.arange(S)
        idxs.append(ix)
    Lmax = max(len(ix) for ix in idxs)
    LPAD = max(2 * P, ((Lmax + P - 1) // P) * P)
    KT = LPAD // P

    in_maps = []
    for c in range(NCORES):
        b, g = divmod(c, 2)
        ix = idxs[b]
        L = len(ix)
        kc = np.zeros((LPAD, DMODEL), np.float32)
        kc[:L] = key[b][ix]
        vc = np.zeros((LPAD, DMODEL), np.float32)
        vc[:L] = value[b][ix]
        # -4.5 recenters exp so fp8 e2 stays below e4m3 max 240 (scores
        # reach ~9.3, exp overflows past 4.5+ln(240)=9.98); the shift is
        # uniform over keys so it cancels in normalization
        mb = np.full((LPAD,), -1e30, np.float32)
        mb[:L] = -4.5
        qrows = query[b, g * SQ:(g + 1) * SQ]
        in_maps.append({
            "qT": f8(qrows.T),
            "kT": f8(kc.T),
            "vT": f8(vc.T),
            "resid": np.ascontiguousarray(qrows),
            "wqT": wqT, "wkT": wkT, "wvT": wvT, "woT": woT,
            "maskb": np.ascontiguousarray(mb.reshape(KT, P).T),
            "gamma": gamma, "beta": beta,
        })
    return in_maps, LPAD


def gather_out(results):
    out = np.empty((B, S, DMODEL), np.float32)
    for c in range(NCORES):
        b, g = divmod(c, 2)
        out[b, g * SQ:(g + 1) * SQ] = results[c]["out"]
    return out


def kernel(query, key, value, mask, Wq, Wk, Wv, Wo, ln_gamma, ln_beta):
    in_maps, LPAD = make_in_maps(query, key, value, mask, Wq, Wk, Wv, Wo,
                                 ln_gamma, ln_beta)
    nc = _get_nc(LPAD)
    res = run_bass_kernel_spmd(nc, in_maps, list(range(NCORES)))
    return gather_out(res.results)
